# revision 10
# baseline (speedup 1.0000x reference)
"""AttentionMemoryFusion kernel for 8 TRN2 NeuronCores (Bass/Tile, SPMD).

Math (refactored from the reference):
  q      = cf @ Wq.T + bq                      [B, HD]
  keys   = mem @ Wk.T          (bk drops out of softmax — per-row const)
  s      = (q @ keys.T) / sqrt(HD)             [B, M]
  w      = exp(s)   (no max subtraction needed: |s| <~ 1)
  ctx    = (w @ mem) / rowsum(w)               [B, D]
  fused  = cf @ Wo1.T + ctx @ (Wo2 @ Wv).T + (Wo2 @ bv + bo)   [B, D]
  mem_new= mem with rows 0..B-1 replaced by cf

Sharding: memory rows split 8 ways (8192 rows/core). Each core computes
flash-style partial ctx^T + partial softmax denominator over its shard.
Per B-half, a ReduceScatter(add) over a B-major [512, 513] buffer hands
core c the fully-reduced ctx rows for its 64-row B-block (plus the
denominator column); each core then runs the output projection for its
block. The first ReduceScatter overlaps the second half's compute.
mem_new is streamed through SBUF; the scatter of cf into rows 0..1023
happens on device via the per-core "head" input.

memory^T (needed for the keys projection, which contracts over D) is
produced by bouncing the bf16 copy of the shard through DRAM and reading
it back with the X-bar transpose DMA (2-byte dtype only) — far cheaper
than PE-transposing 16 MiB through the systolic array.

All tile pools live in one top-level scope: nested (stacked) pools
create released-zone dependencies that serialize phases against each
other; flat pools let the Tile scheduler overlap phases purely by data
dependency.
"""

import sys

if "/opt/trn_rl_repo" not in sys.path:
    sys.path.insert(0, "/opt/trn_rl_repo")

import numpy as np

import concourse.bass as bass
import concourse.tile as tile
from concourse import bacc, mybir
from concourse import bass_utils
from concourse.masks import make_identity

B, D, M, HD = 1024, 512, 65536, 64
NC = 8
MS = M // NC            # 8192 memory rows per core
NCH = MS // 128         # 64 m-chunks of 128 rows
NBLK = 8                # m-blocks of 1024 rows (transpose-DMA granularity)
DC = D // 128           # 4 D-chunks
HEAD = B
F32 = mybir.dt.float32
BF16 = mybir.dt.bfloat16
SCALE = 1.0 / float(np.sqrt(HD, dtype=np.float32))

_CACHE = {}


def _build_module():
    nc = bacc.Bacc("TRN2", target_bir_lowering=False, debug=False, num_devices=NC)

    mem_in = nc.dram_tensor("mem", [MS, D], F32, kind="ExternalInput").ap()
    head_in = nc.dram_tensor("head", [HEAD, D], F32, kind="ExternalInput").ap()
    cfT_in = nc.dram_tensor("cfT", [D, B], F32, kind="ExternalInput").ap()
    cfTs_in = nc.dram_tensor("cfTs", [D, 128], F32, kind="ExternalInput").ap()
    wqT_in = nc.dram_tensor("wqT", [D, HD], F32, kind="ExternalInput").ap()
    wkT_in = nc.dram_tensor("wkT", [D, HD], F32, kind="ExternalInput").ap()
    wovT_in = nc.dram_tensor("wovT", [D, D], F32, kind="ExternalInput").ap()
    wo1T_in = nc.dram_tensor("wo1T", [D, D], F32, kind="ExternalInput").ap()
    bq_in = nc.dram_tensor("bq", [HD, 1], F32, kind="ExternalInput").ap()
    bias2_in = nc.dram_tensor("bias2", [D, 1], F32, kind="ExternalInput").ap()

    mem_out = nc.dram_tensor("mem_out", [MS, D], F32, kind="ExternalOutput").ap()
    fusedT_out = nc.dram_tensor("fusedT_out", [D, 128], F32, kind="ExternalOutput").ap()

    with tile.TileContext(nc) as tc:
        with (
            tc.tile_pool(name="res", bufs=1) as res,
            tc.tile_pool(name="dram", bufs=1, space="DRAM") as dram,
            tc.tile_pool(name="work", bufs=1) as work,
            tc.tile_pool(name="ps", bufs=1, space="PSUM") as ps,
        ):
            # ---- resident SBUF tensors ----
            mem_bf = res.tile([128, NCH * D], BF16)      # natural mem, bf16
            keysT_bf = res.tile([HD, MS], BF16)          # keys^T
            qT_bf = res.tile([HD, B], BF16)
            cfT_bf = res.tile([128, DC * B], BF16)
            cfTs_bf = res.tile([128, D], BF16)
            wkT_bf = res.tile([128, DC * HD], BF16)
            wqT_bf = res.tile([128, DC * HD], BF16)
            wovT_bf = res.tile([128, DC * D], BF16)
            wo1T_bf = res.tile([128, DC * D], BF16)
            ident_bf = res.tile([128, 128], BF16)
            ident_f = res.tile([128, 128], F32)
            ones_bf = res.tile([128, 1], BF16)
            bq_sb = res.tile([HD, 1], F32)
            bias2_sb = res.tile([128, DC], F32)
            den_sb = res.tile([128, B // 2], F32)

            # per-m-block bf16 DRAM bounce (separate tiles keep the
            # transpose-DMA reads from depending on later blocks' writes)
            mbd = [
                dram.tile([1024, D], BF16, name=f"mbd{k}", tag=f"mbd{k}")
                for k in range(NBLK)
            ]
            rs_in = [
                dram.tile([B // 2, D + 1], F32, name=f"rsin{k}", tag=f"rsin{k}")
                for k in range(2)
            ]
            rs_out = [
                dram.tile([B // 2 // NC, D + 1], F32, name=f"rsout{k}", tag=f"rsout{k}")
                for k in range(2)
            ]

            make_identity(nc, ident_bf[:])
            make_identity(nc, ident_f[:])
            nc.gpsimd.memset(ones_bf[:], 1.0)
            nc.gpsimd.memset(den_sb[:], 0.0)
            nc.sync.dma_start(bq_sb[:], bq_in[:])
            for d in range(DC):
                nc.sync.dma_start(bias2_sb[:, d : d + 1], bias2_in[128 * d : 128 * (d + 1), :])

            # ---- load + cast small weights (before the big mem stream) ----
            for d in range(DC):
                st = work.tile([128, B], F32, name="st", tag="stage", bufs=3)
                nc.sync.dma_start(st[:], cfT_in[128 * d : 128 * (d + 1), :])
                nc.vector.tensor_copy(cfT_bf[:, B * d : B * (d + 1)], st[:])
            st = work.tile([128, B], F32, name="st", tag="stage", bufs=3)
            for d in range(DC):
                nc.sync.dma_start(
                    st[:, 128 * d : 128 * (d + 1)], cfTs_in[128 * d : 128 * (d + 1), :]
                )
                nc.sync.dma_start(
                    st[:, 512 + HD * d : 512 + HD * (d + 1)],
                    wkT_in[128 * d : 128 * (d + 1), :],
                )
                nc.sync.dma_start(
                    st[:, 768 + HD * d : 768 + HD * (d + 1)],
                    wqT_in[128 * d : 128 * (d + 1), :],
                )
            nc.vector.tensor_copy(cfTs_bf[:], st[:, 0:512])
            nc.vector.tensor_copy(wkT_bf[:], st[:, 512:768])
            nc.vector.tensor_copy(wqT_bf[:], st[:, 768:1024])
            for d in range(DC):
                st = work.tile([128, B], F32, name="st", tag="stage", bufs=3)
                nc.sync.dma_start(st[:, 0:D], wovT_in[128 * d : 128 * (d + 1), :])
                nc.sync.dma_start(st[:, D : 2 * D], wo1T_in[128 * d : 128 * (d + 1), :])
                nc.vector.tensor_copy(wovT_bf[:, D * d : D * (d + 1)], st[:, 0:D])
                nc.vector.tensor_copy(wo1T_bf[:, D * d : D * (d + 1)], st[:, D : 2 * D])

            # ---- head rows of mem_new (scatter target) ----
            nc.sync.dma_start(mem_out[0:HEAD, :], head_in[:])

            # ---- q^T = Wq @ cf^T + bq  (two N=512 halves) ----
            for h in range(2):
                qp = ps.tile([128, 512], F32, name="qp", tag="scp", bufs=3)
                for d in range(DC):
                    nc.tensor.matmul(
                        qp[0:HD, :],
                        wqT_bf[:, HD * d : HD * (d + 1)],
                        cfT_bf[:, B * d + 512 * h : B * d + 512 * (h + 1)],
                        start=(d == 0),
                        stop=(d == DC - 1),
                    )
                nc.vector.tensor_scalar_add(
                    qT_bf[:, 512 * h : 512 * (h + 1)], qp[0:HD, :], bq_sb[:]
                )

            # ---- phase 1: stream shard; writeback; cast; keys^T ----
            for mb in range(NBLK):
                for j in range(8):
                    i = 8 * mb + j
                    mc = work.tile([128, D], F32, name="mc", tag="ld", bufs=6)
                    nc.sync.dma_start(mc[:], mem_in[128 * i : 128 * (i + 1), :])
                    if i >= HEAD // 128:
                        nc.sync.dma_start(mem_out[128 * i : 128 * (i + 1), :], mc[:])
                    nc.vector.tensor_copy(mem_bf[:, D * i : D * (i + 1)], mc[:])
                    nc.sync.dma_start(
                        mbd[mb][128 * j : 128 * (j + 1), :], mem_bf[:, D * i : D * (i + 1)]
                    )
                mts = []
                for d in range(DC):
                    mt = work.tile([128, 1024], BF16, name="mt", tag=f"memT{d}", bufs=2)
                    nc.sync.dma_start(
                        mt[:], mbd[mb][:, 128 * d : 128 * (d + 1)], transpose=True
                    )
                    mts.append(mt)
                for t in range(2):
                    kt = ps.tile([128, 512], F32, name="kt", tag="scp", bufs=3)
                    for d in range(DC):
                        nc.tensor.matmul(
                            kt[0:HD, :],
                            wkT_bf[:, HD * d : HD * (d + 1)],
                            mts[d][:, 512 * t : 512 * (t + 1)],
                            start=(d == 0),
                            stop=(d == DC - 1),
                        )
                    nc.vector.tensor_copy(
                        keysT_bf[:, 1024 * mb + 512 * t : 1024 * mb + 512 * (t + 1)],
                        kt[0:HD, :],
                    )

            # ---- phase 2: per B-half flash pass -> B-major bounce -> RS ----
            for h in range(2):
                ctx_ps = ps.tile([128, DC * 512], F32, name="ctx_ps", tag="ctxp", bufs=1)
                wsum = work.tile([128, 512], F32, name="wsum", tag="wsum", bufs=2)
                for i in range(NCH):
                    sc = ps.tile([128, 512], F32, name="sc", tag="scp", bufs=3)
                    nc.tensor.matmul(
                        sc[:],
                        keysT_bf[:, 128 * i : 128 * (i + 1)],
                        qT_bf[:, 512 * h : 512 * (h + 1)],
                        start=True,
                        stop=True,
                    )
                    wt = work.tile([128, 512], BF16, name="wt", tag="wT", bufs=4)
                    nc.scalar.activation(
                        wt[:], sc[:], mybir.ActivationFunctionType.Exp, scale=SCALE
                    )
                    if i == 0:
                        nc.vector.tensor_copy(wsum[:], wt[:])
                    else:
                        nc.vector.tensor_add(wsum[:], wsum[:], wt[:])
                    for d in range(DC):
                        nc.tensor.matmul(
                            ctx_ps[:, 512 * d : 512 * (d + 1)],
                            mem_bf[:, D * i + 128 * d : D * i + 128 * (d + 1)],
                            wt[:],
                            start=(i == 0),
                            stop=(i == NCH - 1),
                            skip_group_check=True,
                        )
                # denominator: ones^T @ wsum  (partition reduction)
                wsum_bf = work.tile([128, 512], BF16, name="wsum_bf", tag="wsumbf", bufs=2)
                nc.vector.tensor_copy(wsum_bf[:], wsum[:])
                dp = ps.tile([1, 512], F32, name="dp", tag="sm", bufs=1)
                nc.tensor.matmul(dp[:], ones_bf[:], wsum_bf[:], start=True, stop=True)
                nc.scalar.copy(den_sb[0:1, :], dp[:])

                ctx_sb = work.tile([128, DC * 512], F32, name="ctx_sb", tag="ctxsb", bufs=1)
                nc.scalar.copy(ctx_sb[:], ctx_ps[:])

                # transpose ctx^T -> B-major, append denominator column
                for b in range(4):
                    nat = work.tile([128, D + 1], F32, name="nat", tag="nat", bufs=2)
                    for d in range(DC):
                        tp = ps.tile([128, 128], F32, name="tp", tag="sm", bufs=1)
                        nc.tensor.transpose(
                            tp[:],
                            ctx_sb[:, 512 * d + 128 * b : 512 * d + 128 * (b + 1)],
                            ident_f[:],
                        )
                        nc.scalar.copy(nat[:, 128 * d : 128 * (d + 1)], tp[:])
                    tp = ps.tile([128, 128], F32, name="tp", tag="sm", bufs=1)
                    nc.tensor.transpose(tp[:], den_sb[:, 128 * b : 128 * (b + 1)], ident_f[:])
                    nc.vector.tensor_copy(nat[:, D : D + 1], tp[:, 0:1])
                    nc.sync.dma_start(rs_in[h][128 * b : 128 * (b + 1), :], nat[:])

                # cross-core reduce for this half (first one overlaps the
                # second half's compute)
                nc.gpsimd.collective_compute(
                    "ReduceScatter",
                    mybir.AluOpType.add,
                    replica_groups=[list(range(NC))],
                    ins=[rs_in[h][:].opt()],
                    outs=[rs_out[h][:].opt()],
                )

            # ---- phase 3: epilogue for this core's two 64-row B-blocks ----
            for h in range(2):
                ctxn_in = work.tile([64, D + 1], F32, name="ctxn_in", tag="ep_in", bufs=2)
                nc.sync.dma_start(ctxn_in[:], rs_out[h][:])
                recip = work.tile([64, 1], F32, name="recip", tag="ep_r", bufs=2)
                nc.vector.reciprocal(recip[:], ctxn_in[:, D : D + 1])
                ctxn_bf = work.tile([64, D], BF16, name="ctxn_bf", tag="ep_nbf", bufs=2)
                nc.vector.tensor_scalar_mul(ctxn_bf[:], ctxn_in[:, 0:D], recip[:])
                ctxnT_bf = work.tile(
                    [128, DC * 64], BF16, name="ctxnT_bf", tag="ep_tbf", bufs=2
                )
                for d in range(DC):
                    tp = ps.tile([128, 64], BF16, name="tpc", tag="sm", bufs=1)
                    nc.tensor.transpose(
                        tp[:], ctxn_bf[:, 128 * d : 128 * (d + 1)], ident_bf[0:64, 0:64]
                    )
                    nc.scalar.copy(ctxnT_bf[:, 64 * d : 64 * (d + 1)], tp[:])
                fused_sb = work.tile([128, D], F32, name="fused_sb", tag="ep_out", bufs=1)
                for do in range(DC):
                    fps = ps.tile([128, 64], F32, name="fps", tag="sm", bufs=1)
                    for k in range(DC):
                        nc.tensor.matmul(
                            fps[:],
                            wovT_bf[:, D * k + 128 * do : D * k + 128 * (do + 1)],
                            ctxnT_bf[:, 64 * k : 64 * (k + 1)],
                            start=(k == 0),
                            stop=False,
                            skip_group_check=True,
                        )
                    for k in range(DC):
                        nc.tensor.matmul(
                            fps[:],
                            wo1T_bf[:, D * k + 128 * do : D * k + 128 * (do + 1)],
                            cfTs_bf[:, 128 * k + 64 * h : 128 * k + 64 * (h + 1)],
                            start=False,
                            stop=(k == DC - 1),
                            skip_group_check=True,
                        )
                    nc.vector.tensor_scalar_add(
                        fused_sb[:, 128 * do + 64 * h : 128 * do + 64 * (h + 1)],
                        fps[:],
                        bias2_sb[:, do : do + 1],
                    )
                    nc.sync.dma_start(
                        fusedT_out[128 * do : 128 * (do + 1), 64 * h : 64 * (h + 1)],
                        fused_sb[:, 128 * do + 64 * h : 128 * do + 64 * (h + 1)],
                    )

    nc.compile()
    return nc


def _get_module():
    if "nc" not in _CACHE:
        _CACHE["nc"] = _build_module()
    return _CACHE["nc"]


def _prepare_in_maps(current_features, memory, Wq, bq, Wk, bk, Wv, bv, Wo, bo):
    cf = np.asarray(current_features, np.float32)
    memory = np.asarray(memory, np.float32)
    Wq, bq = np.asarray(Wq, np.float32), np.asarray(bq, np.float32)
    Wk = np.asarray(Wk, np.float32)
    Wv, bv = np.asarray(Wv, np.float32), np.asarray(bv, np.float32)
    Wo, bo = np.asarray(Wo, np.float32), np.asarray(bo, np.float32)

    Wo1, Wo2 = Wo[:, :D], Wo[:, D:]
    cfT = np.ascontiguousarray(cf.T)
    shared = {
        "cfT": cfT,
        "wqT": np.ascontiguousarray(Wq.T),
        "wkT": np.ascontiguousarray(Wk.T),
        "wovT": np.ascontiguousarray((Wo2 @ Wv).T),
        "wo1T": np.ascontiguousarray(Wo1.T),
        "bq": np.ascontiguousarray(bq.reshape(HD, 1)),
        "bias2": np.ascontiguousarray((Wo2 @ bv + bo).reshape(D, 1)),
    }
    in_maps = []
    for c in range(NC):
        m = dict(shared)
        m["mem"] = np.ascontiguousarray(memory[MS * c : MS * (c + 1)])
        m["head"] = cf if c == 0 else np.ascontiguousarray(memory[MS * c : MS * c + HEAD])
        # cols 0:64 -> B-rows 64c..64c+64 (half 0); cols 64:128 -> 512+64c.. (half 1)
        m["cfTs"] = np.ascontiguousarray(
            np.concatenate(
                [cfT[:, 64 * c : 64 * (c + 1)], cfT[:, 512 + 64 * c : 512 + 64 * (c + 1)]],
                axis=1,
            )
        )
        in_maps.append(m)
    return in_maps


def _assemble(res):
    fusedT = np.empty((D, B), np.float32)
    for c in range(NC):
        out = res.results[c]["fusedT_out"]
        fusedT[:, 64 * c : 64 * (c + 1)] = out[:, 0:64]
        fusedT[:, 512 + 64 * c : 512 + 64 * (c + 1)] = out[:, 64:128]
    mem_new = np.concatenate([res.results[c]["mem_out"] for c in range(NC)], axis=0)
    return np.ascontiguousarray(fusedT.T), mem_new


def kernel(**inputs):
    in_maps = _prepare_in_maps(**inputs)
    nc = _get_module()
    res = bass_utils.run_bass_kernel_spmd(nc, in_maps, core_ids=list(range(NC)))
    return _assemble(res)


def run_traced(**inputs):
    in_maps = _prepare_in_maps(**inputs)
    nc = _get_module()
    res = bass_utils.run_bass_kernel_spmd(
        nc, in_maps, core_ids=list(range(NC)), trace=True
    )
    res.outputs = _assemble(res)
    return res


# revision 11
# speedup vs baseline: 1.1276x; 1.1276x over previous
"""AttentionMemoryFusion kernel for 8 TRN2 NeuronCores (Bass/Tile, SPMD).

Math (refactored from the reference):
  q      = cf @ Wq.T + bq                      [B, HD]
  keys   = mem @ Wk.T          (bk drops out of softmax — per-row const)
  s      = (q @ keys.T) / sqrt(HD)             [B, M]
  w      = exp(s)   (no max subtraction needed: |s| <~ 1)
  ctx    = (w @ mem) / rowsum(w)               [B, D]
  fused  = cf @ Wo1.T + ctx @ (Wo2 @ Wv).T + (Wo2 @ bv + bo)   [B, D]
  mem_new= mem with rows 0..B-1 replaced by cf

Sharding: memory rows split 8 ways (8192 rows/core). Each core computes
flash-style partial ctx^T + partial softmax denominator over its shard.
Per B-half, a ReduceScatter(add) over a B-major [512, 513] buffer hands
core c the fully-reduced ctx rows for its 64-row B-block (plus the
denominator column); each core then runs the output projection for its
block. The first ReduceScatter overlaps the second half's compute.
mem_new is streamed through SBUF; the scatter of cf into rows 0..1023
happens on device via the per-core "head" input.

memory^T (needed for the keys projection, which contracts over D) is
produced by bouncing the bf16 copy of the shard through DRAM and reading
it back with the X-bar transpose DMA (2-byte dtype only) — far cheaper
than PE-transposing 16 MiB through the systolic array.

All tile pools live in one top-level scope: nested (stacked) pools
create released-zone dependencies that serialize phases against each
other; flat pools let the Tile scheduler overlap phases purely by data
dependency.
"""

import sys

if "/opt/trn_rl_repo" not in sys.path:
    sys.path.insert(0, "/opt/trn_rl_repo")

import numpy as np

import concourse.bass as bass
import concourse.tile as tile
from concourse import bacc, mybir
from concourse import bass_utils
from concourse.masks import make_identity

B, D, M, HD = 1024, 512, 65536, 64
NC = 8
MS = M // NC            # 8192 memory rows per core
NCH = MS // 128         # 64 m-chunks of 128 rows
NBLK = 8                # m-blocks of 1024 rows (transpose-DMA granularity)
DC = D // 128           # 4 D-chunks
HEAD = B
F32 = mybir.dt.float32
BF16 = mybir.dt.bfloat16
SCALE = 1.0 / float(np.sqrt(HD, dtype=np.float32))

_CACHE = {}


def _build_module():
    nc = bacc.Bacc("TRN2", target_bir_lowering=False, debug=False, num_devices=NC)

    mem_in = nc.dram_tensor("mem", [MS, D], F32, kind="ExternalInput").ap()
    head_in = nc.dram_tensor("head", [HEAD, D], F32, kind="ExternalInput").ap()
    cfT_in = nc.dram_tensor("cfT", [D, B], F32, kind="ExternalInput").ap()
    cfTs_in = nc.dram_tensor("cfTs", [D, 128], F32, kind="ExternalInput").ap()
    wqT_in = nc.dram_tensor("wqT", [D, HD], F32, kind="ExternalInput").ap()
    wkT_in = nc.dram_tensor("wkT", [D, HD], F32, kind="ExternalInput").ap()
    wovT_in = nc.dram_tensor("wovT", [D, D], F32, kind="ExternalInput").ap()
    wo1T_in = nc.dram_tensor("wo1T", [D, D], F32, kind="ExternalInput").ap()
    bq_in = nc.dram_tensor("bq", [HD, 1], F32, kind="ExternalInput").ap()
    bias2_in = nc.dram_tensor("bias2", [D, 1], F32, kind="ExternalInput").ap()

    mem_out = nc.dram_tensor("mem_out", [MS, D], F32, kind="ExternalOutput").ap()
    fusedT_out = nc.dram_tensor("fusedT_out", [D, 128], F32, kind="ExternalOutput").ap()

    with tile.TileContext(nc) as tc:
        with (
            tc.tile_pool(name="res", bufs=1) as res,
            tc.tile_pool(name="dram", bufs=1, space="DRAM") as dram,
            tc.tile_pool(name="work", bufs=1) as work,
            tc.tile_pool(name="ps", bufs=1, space="PSUM") as ps,
        ):
            # ---- resident SBUF tensors ----
            mem_bf = res.tile([128, NCH * D], BF16)      # natural mem, bf16
            keysT_bf = res.tile([HD, MS], BF16)          # keys^T
            qT_bf = res.tile([HD, B], BF16)
            cfT_bf = res.tile([128, DC * B], BF16)
            cfTs_bf = res.tile([128, D], BF16)
            wkT_bf = res.tile([128, DC * HD], BF16)
            wqT_bf = res.tile([128, DC * HD], BF16)
            wovT_bf = res.tile([128, DC * D], BF16)
            wo1T_bf = res.tile([128, DC * D], BF16)
            ident_bf = res.tile([128, 128], BF16)
            ident_f = res.tile([128, 128], F32)
            ones_bf = res.tile([128, 1], BF16)
            bq_sb = res.tile([HD, 1], F32)
            bias2_sb = res.tile([128, DC], F32)
            den_sb = res.tile([128, B // 2], F32)

            # per-m-block bf16 DRAM bounce (separate tiles keep the
            # transpose-DMA reads from depending on later blocks' writes)
            mbd = [
                dram.tile([1024, D], BF16, name=f"mbd{k}", tag=f"mbd{k}")
                for k in range(NBLK)
            ]
            rs_in = [
                dram.tile([B // 2, D + 1], F32, name=f"rsin{k}", tag=f"rsin{k}")
                for k in range(2)
            ]
            rs_out = [
                dram.tile([B // 2 // NC, D + 1], F32, name=f"rsout{k}", tag=f"rsout{k}")
                for k in range(2)
            ]

            make_identity(nc, ident_bf[:])
            make_identity(nc, ident_f[:])
            nc.gpsimd.memset(ones_bf[:], 1.0)
            nc.gpsimd.memset(den_sb[:], 0.0)
            nc.sync.dma_start(bq_sb[:], bq_in[:])
            for d in range(DC):
                nc.sync.dma_start(bias2_sb[:, d : d + 1], bias2_in[128 * d : 128 * (d + 1), :])

            # ---- load + cast small weights (before the big mem stream) ----
            for d in range(DC):
                st = work.tile([128, B], F32, name="st", tag="stage", bufs=3)
                nc.sync.dma_start(st[:], cfT_in[128 * d : 128 * (d + 1), :])
                nc.vector.tensor_copy(cfT_bf[:, B * d : B * (d + 1)], st[:])
            st = work.tile([128, B], F32, name="st", tag="stage", bufs=3)
            for d in range(DC):
                nc.sync.dma_start(
                    st[:, 128 * d : 128 * (d + 1)], cfTs_in[128 * d : 128 * (d + 1), :]
                )
                nc.sync.dma_start(
                    st[:, 512 + HD * d : 512 + HD * (d + 1)],
                    wkT_in[128 * d : 128 * (d + 1), :],
                )
                nc.sync.dma_start(
                    st[:, 768 + HD * d : 768 + HD * (d + 1)],
                    wqT_in[128 * d : 128 * (d + 1), :],
                )
            nc.vector.tensor_copy(cfTs_bf[:], st[:, 0:512])
            nc.vector.tensor_copy(wkT_bf[:], st[:, 512:768])
            nc.vector.tensor_copy(wqT_bf[:], st[:, 768:1024])
            for d in range(DC):
                st = work.tile([128, B], F32, name="st", tag="stage", bufs=3)
                nc.sync.dma_start(st[:, 0:D], wovT_in[128 * d : 128 * (d + 1), :])
                nc.sync.dma_start(st[:, D : 2 * D], wo1T_in[128 * d : 128 * (d + 1), :])
                nc.vector.tensor_copy(wovT_bf[:, D * d : D * (d + 1)], st[:, 0:D])
                nc.vector.tensor_copy(wo1T_bf[:, D * d : D * (d + 1)], st[:, D : 2 * D])

            # ---- head rows of mem_new (scatter target) ----
            nc.sync.dma_start(mem_out[0:HEAD, :], head_in[:])

            # ---- q^T = Wq @ cf^T + bq  (two N=512 halves) ----
            for h in range(2):
                qp = ps.tile([128, 512], F32, name="qp", tag="kt", bufs=2)
                for d in range(DC):
                    nc.tensor.matmul(
                        qp[0:HD, :],
                        wqT_bf[:, HD * d : HD * (d + 1)],
                        cfT_bf[:, B * d + 512 * h : B * d + 512 * (h + 1)],
                        start=(d == 0),
                        stop=(d == DC - 1),
                    )
                nc.vector.tensor_scalar_add(
                    qT_bf[:, 512 * h : 512 * (h + 1)], qp[0:HD, :], bq_sb[:]
                )

            # ---- phase 1: stream shard; writeback; cast; keys^T ----
            for mb in range(NBLK):
                for j in range(8):
                    i = 8 * mb + j
                    mc = work.tile([128, D], F32, name="mc", tag="ld", bufs=6)
                    nc.sync.dma_start(mc[:], mem_in[128 * i : 128 * (i + 1), :])
                    if i >= HEAD // 128:
                        nc.sync.dma_start(mem_out[128 * i : 128 * (i + 1), :], mc[:])
                    nc.vector.tensor_copy(mem_bf[:, D * i : D * (i + 1)], mc[:])
                    nc.sync.dma_start(
                        mbd[mb][128 * j : 128 * (j + 1), :], mem_bf[:, D * i : D * (i + 1)]
                    )
                mts = []
                for d in range(DC):
                    mt = work.tile([128, 1024], BF16, name="mt", tag=f"memT{d}", bufs=2)
                    nc.sync.dma_start(
                        mt[:], mbd[mb][:, 128 * d : 128 * (d + 1)], transpose=True
                    )
                    mts.append(mt)
                for t in range(2):
                    kt = ps.tile([128, 512], F32, name="kt", tag="kt", bufs=2)
                    for d in range(DC):
                        nc.tensor.matmul(
                            kt[0:HD, :],
                            wkT_bf[:, HD * d : HD * (d + 1)],
                            mts[d][:, 512 * t : 512 * (t + 1)],
                            start=(d == 0),
                            stop=(d == DC - 1),
                        )
                    nc.vector.tensor_copy(
                        keysT_bf[:, 1024 * mb + 512 * t : 1024 * mb + 512 * (t + 1)],
                        kt[0:HD, :],
                    )

            # ---- phase 2: per B-half flash pass -> B-major bounce -> RS ----
            for h in range(2):
                ctx_ps = ps.tile([128, DC * 512], F32, name="ctx_ps", tag="ctxp", bufs=1)
                wsum = work.tile([128, 512], F32, name="wsum", tag="wsum", bufs=2)
                for i in range(NCH):
                    sc = ps.tile([128, 512], F32, name="sc", tag="sc", bufs=2)
                    nc.tensor.matmul(
                        sc[:],
                        keysT_bf[:, 128 * i : 128 * (i + 1)],
                        qT_bf[:, 512 * h : 512 * (h + 1)],
                        start=True,
                        stop=True,
                    )
                    wt = work.tile([128, 512], BF16, name="wt", tag="wT", bufs=4)
                    nc.scalar.activation(
                        wt[:], sc[:], mybir.ActivationFunctionType.Exp, scale=SCALE
                    )
                    if i == 0:
                        nc.vector.tensor_copy(wsum[:], wt[:])
                    else:
                        nc.vector.tensor_add(wsum[:], wsum[:], wt[:])
                    for d in range(DC):
                        nc.tensor.matmul(
                            ctx_ps[:, 512 * d : 512 * (d + 1)],
                            mem_bf[:, D * i + 128 * d : D * i + 128 * (d + 1)],
                            wt[:],
                            start=(i == 0),
                            stop=(i == NCH - 1),
                            skip_group_check=True,
                        )
                # denominator: ones^T @ wsum  (partition reduction)
                wsum_bf = work.tile([128, 512], BF16, name="wsum_bf", tag="wsumbf", bufs=2)
                nc.vector.tensor_copy(wsum_bf[:], wsum[:])
                dp = ps.tile([1, 512], F32, name="dp", tag="kt", bufs=2)
                nc.tensor.matmul(dp[:], ones_bf[:], wsum_bf[:], start=True, stop=True)
                nc.scalar.copy(den_sb[0:1, :], dp[:])

                ctx_sb = work.tile([128, DC * 512], F32, name="ctx_sb", tag="ctxsb", bufs=1)
                nc.scalar.copy(ctx_sb[:], ctx_ps[:])

                # transpose ctx^T -> B-major, append denominator column
                for b in range(4):
                    nat = work.tile([128, D + 1], F32, name="nat", tag="nat", bufs=2)
                    for d in range(DC):
                        tp = ps.tile([128, 128], F32, name="tp", tag="kt", bufs=2)
                        nc.tensor.transpose(
                            tp[:],
                            ctx_sb[:, 512 * d + 128 * b : 512 * d + 128 * (b + 1)],
                            ident_f[:],
                        )
                        nc.scalar.copy(nat[:, 128 * d : 128 * (d + 1)], tp[:])
                    tp = ps.tile([128, 128], F32, name="tp", tag="kt", bufs=2)
                    nc.tensor.transpose(tp[:], den_sb[:, 128 * b : 128 * (b + 1)], ident_f[:])
                    nc.vector.tensor_copy(nat[:, D : D + 1], tp[:, 0:1])
                    nc.sync.dma_start(rs_in[h][128 * b : 128 * (b + 1), :], nat[:])

                # cross-core reduce for this half (first one overlaps the
                # second half's compute)
                nc.gpsimd.collective_compute(
                    "ReduceScatter",
                    mybir.AluOpType.add,
                    replica_groups=[list(range(NC))],
                    ins=[rs_in[h][:].opt()],
                    outs=[rs_out[h][:].opt()],
                )

            # ---- phase 3: epilogue for this core's two 64-row B-blocks ----
            for h in range(2):
                ctxn_in = work.tile([64, D + 1], F32, name="ctxn_in", tag="ep_in", bufs=2)
                nc.sync.dma_start(ctxn_in[:], rs_out[h][:])
                recip = work.tile([64, 1], F32, name="recip", tag="ep_r", bufs=2)
                nc.vector.reciprocal(recip[:], ctxn_in[:, D : D + 1])
                ctxn_bf = work.tile([64, D], BF16, name="ctxn_bf", tag="ep_nbf", bufs=2)
                nc.vector.tensor_scalar_mul(ctxn_bf[:], ctxn_in[:, 0:D], recip[:])
                ctxnT_bf = work.tile(
                    [128, DC * 64], BF16, name="ctxnT_bf", tag="ep_tbf", bufs=2
                )
                for d in range(DC):
                    tp = ps.tile([128, 64], BF16, name="tpc", tag="kt", bufs=2)
                    nc.tensor.transpose(
                        tp[:], ctxn_bf[:, 128 * d : 128 * (d + 1)], ident_bf[0:64, 0:64]
                    )
                    nc.scalar.copy(ctxnT_bf[:, 64 * d : 64 * (d + 1)], tp[:])
                fused_sb = work.tile([128, D], F32, name="fused_sb", tag="ep_out", bufs=1)
                for do in range(DC):
                    fps = ps.tile([128, 64], F32, name="fps", tag="kt", bufs=2)
                    for k in range(DC):
                        nc.tensor.matmul(
                            fps[:],
                            wovT_bf[:, D * k + 128 * do : D * k + 128 * (do + 1)],
                            ctxnT_bf[:, 64 * k : 64 * (k + 1)],
                            start=(k == 0),
                            stop=False,
                            skip_group_check=True,
                        )
                    for k in range(DC):
                        nc.tensor.matmul(
                            fps[:],
                            wo1T_bf[:, D * k + 128 * do : D * k + 128 * (do + 1)],
                            cfTs_bf[:, 128 * k + 64 * h : 128 * k + 64 * (h + 1)],
                            start=False,
                            stop=(k == DC - 1),
                            skip_group_check=True,
                        )
                    nc.vector.tensor_scalar_add(
                        fused_sb[:, 128 * do + 64 * h : 128 * do + 64 * (h + 1)],
                        fps[:],
                        bias2_sb[:, do : do + 1],
                    )
                    nc.sync.dma_start(
                        fusedT_out[128 * do : 128 * (do + 1), 64 * h : 64 * (h + 1)],
                        fused_sb[:, 128 * do + 64 * h : 128 * do + 64 * (h + 1)],
                    )

    nc.compile()
    return nc


def _get_module():
    if "nc" not in _CACHE:
        _CACHE["nc"] = _build_module()
    return _CACHE["nc"]


def _prepare_in_maps(current_features, memory, Wq, bq, Wk, bk, Wv, bv, Wo, bo):
    cf = np.asarray(current_features, np.float32)
    memory = np.asarray(memory, np.float32)
    Wq, bq = np.asarray(Wq, np.float32), np.asarray(bq, np.float32)
    Wk = np.asarray(Wk, np.float32)
    Wv, bv = np.asarray(Wv, np.float32), np.asarray(bv, np.float32)
    Wo, bo = np.asarray(Wo, np.float32), np.asarray(bo, np.float32)

    Wo1, Wo2 = Wo[:, :D], Wo[:, D:]
    cfT = np.ascontiguousarray(cf.T)
    shared = {
        "cfT": cfT,
        "wqT": np.ascontiguousarray(Wq.T),
        "wkT": np.ascontiguousarray(Wk.T),
        "wovT": np.ascontiguousarray((Wo2 @ Wv).T),
        "wo1T": np.ascontiguousarray(Wo1.T),
        "bq": np.ascontiguousarray(bq.reshape(HD, 1)),
        "bias2": np.ascontiguousarray((Wo2 @ bv + bo).reshape(D, 1)),
    }
    in_maps = []
    for c in range(NC):
        m = dict(shared)
        m["mem"] = np.ascontiguousarray(memory[MS * c : MS * (c + 1)])
        m["head"] = cf if c == 0 else np.ascontiguousarray(memory[MS * c : MS * c + HEAD])
        # cols 0:64 -> B-rows 64c..64c+64 (half 0); cols 64:128 -> 512+64c.. (half 1)
        m["cfTs"] = np.ascontiguousarray(
            np.concatenate(
                [cfT[:, 64 * c : 64 * (c + 1)], cfT[:, 512 + 64 * c : 512 + 64 * (c + 1)]],
                axis=1,
            )
        )
        in_maps.append(m)
    return in_maps


def _assemble(res):
    fusedT = np.empty((D, B), np.float32)
    for c in range(NC):
        out = res.results[c]["fusedT_out"]
        fusedT[:, 64 * c : 64 * (c + 1)] = out[:, 0:64]
        fusedT[:, 512 + 64 * c : 512 + 64 * (c + 1)] = out[:, 64:128]
    mem_new = np.concatenate([res.results[c]["mem_out"] for c in range(NC)], axis=0)
    return np.ascontiguousarray(fusedT.T), mem_new


def kernel(**inputs):
    in_maps = _prepare_in_maps(**inputs)
    nc = _get_module()
    res = bass_utils.run_bass_kernel_spmd(nc, in_maps, core_ids=list(range(NC)))
    return _assemble(res)


def run_traced(**inputs):
    in_maps = _prepare_in_maps(**inputs)
    nc = _get_module()
    res = bass_utils.run_bass_kernel_spmd(
        nc, in_maps, core_ids=list(range(NC)), trace=True
    )
    res.outputs = _assemble(res)
    return res
